# revision 36
# baseline (speedup 1.0000x reference)
"""Mixtral-style MoE block (T=2048, H=1024, F=2048, E=8, top-2) on 8 trn2
NeuronCores.

Expert-parallel with host-side dispatch: the router (a [2048,8] matmul +
softmax + top-2) is computed on host in fp32, and each core receives only
the tokens routed to its expert, capacity-padded to C (= max expert load
rounded up). The core computes its expert's SwiGLU FFN in bf16 (fp32 PSUM
accumulation), scales rows by the renormalized top-2 combine weight, and
writes its [C, H] partial; the host scatter-adds the two partials per
token. No collectives. bf16 halves both Tensor rows and HBM bytes vs
fp32r; sparse dispatch cuts matmul work 2048/C (~3.6x) vs dense.
"""
import numpy as np
import ml_dtypes

try:
    import concourse  # noqa: F401
except ImportError:  # pragma: no cover
    import sys
    sys.path.insert(0, "/opt/trn_rl_repo")

from concourse import mybir, bacc
import concourse.tile as tile
from concourse.bass_utils import run_bass_kernel_spmd

T, H, F, E, TOP_K = 2048, 1024, 2048, 8, 2
P = 128
KH = H // P          # 8 k-tiles over H
KF = F // P          # 16 f-tiles over F
F32 = mybir.dt.float32
BF16 = mybir.dt.bfloat16
BF16NP = ml_dtypes.bfloat16
PSUM = "PSUM"

_NC_CACHE = {}


def _route(hidden_states, gate_w):
    """Host router, replicating reference: softmax fp32 -> top-2 (ties to
    lower index, like lax.top_k) -> renormalize."""
    x = np.asarray(hidden_states, np.float32)
    logits = x @ np.asarray(gate_w, np.float32)
    logits = logits - logits.max(-1, keepdims=True)
    p = np.exp(logits)
    p /= p.sum(-1, keepdims=True)
    idx = np.argsort(-p, axis=-1, kind="stable")[:, :TOP_K]
    tw = np.take_along_axis(p, idx, -1)
    tw = (tw / tw.sum(-1, keepdims=True)).astype(np.float32)
    toks, wts = [], []
    for e in range(E):
        sel = idx == e                      # [T, K]; <=1 hit per token
        t = np.nonzero(sel.any(1))[0]
        w = np.where(sel[:, 0], tw[:, 0], tw[:, 1])[t]
        toks.append(t)
        wts.append(w.astype(np.float32))
    return toks, wts


NCH = 3  # phase-A PSUM chunks (2*NCH+1 banks <= 8)


def _cap(toks):
    cap = max(len(t) for t in toks)
    return max(((cap + 8 * NCH - 1) // (8 * NCH)) * (8 * NCH), NCH * P)


def _chunks(C):
    """C tokens in NCH equal 8-aligned PSUM chunks; small chunks let compute
    start as soon as the first lands."""
    s = C // NCH
    return s, [(i * s, (i + 1) * s) for i in range(NCH)]


def build(C):
    MT = -(-C // P)                     # phase-B m-tiles (last may be partial)
    MS = [P] * (C // P) + ([C % P] if C % P else [])
    s, chunks = _chunks(C)

    nc = bacc.Bacc("TRN2", target_bir_lowering=False, debug=False,
                   num_devices=E)
    # chunk-major layout: per-partition lines are contiguous (KH*s*2 bytes),
    # so each chunk is one efficient DMA with large descriptors
    xg = nc.dram_tensor("xg", [NCH, P, KH, C // NCH], BF16,
                        kind="ExternalInput")
    w13 = nc.dram_tensor("w13", [P, KF, 2, KH, P], BF16, kind="ExternalInput")
    w2d = nc.dram_tensor("w2d", [2, P, KF, H // 2], BF16,
                         kind="ExternalInput")
    cwd = nc.dram_tensor("cwd", [P, MT], F32, kind="ExternalInput")
    oute = nc.dram_tensor("oute", [C, H], F32, kind="ExternalOutput")

    with tile.TileContext(nc) as tc:
        with (
            tc.tile_pool(name="big", bufs=1) as big,
            tc.tile_pool(name="small", bufs=1) as small,
            tc.tile_pool(name="wpool", bufs=3) as wpool,
            tc.tile_pool(name="evac", bufs=4) as evac,
        ):
            # PE p-state warm-up input: memset on the otherwise-idle Vector
            # engine so it does not queue behind DMA issues
            warm_sb = small.tile([P, 256], BF16)
            nc.vector.memset(warm_sb[:], 0.0)
            # token shards: one tile per PSUM chunk so compute starts on
            # chunk 0 as soon as its DMA lands. One DMA per chunk: each
            # dma_start issue costs ~0.65us serially on its engine, so fewer
            # larger DMAs win at startup
            # c0/c1 land in parallel on separate queues; c2 trails on sync
            xg_s = []
            xg_eng = [nc.sync, nc.gpsimd, nc.sync]
            for i in range(NCH):
                xt = big.tile([P, KH, s], BF16, name=f"xg{i}")
                xg_eng[i % 3].dma_start(out=xt[:], in_=xg.ap()[i])
                xg_s.append(xt)
            cw_s = small.tile([P, MT], F32)
            nc.sync.dma_start(out=cw_s[:], in_=cwd.ap())
            # w2 resident in SBUF (4MB bf16) in two H-halves: half 0 streams
            # during late phase A, half 1 during phase B's first n-pass,
            # keeping w2 out of the DMA-balanced phase-A startup window
            w2_s = [big.tile([P, KF, H // 2], BF16, name=f"w2h{n}")
                    for n in range(2)]
            inter = big.tile([P, KF, C], BF16)  # inter[f%P, f//P, tok]

            # Phase A: inter[f, t] = silu(w1.T x)[f, t] * (w3.T x)[f, t]
            # per f-tile: all w1 chunk-groups first, then all w3 groups, so
            # the w3 weight half may arrive ~2us later than the w1 half
            with tc.tile_pool(name="psA", bufs=1, space=PSUM) as psA:
                # dummy accumulating matmuls keep the Tensor engine busy (and
                # its p-state ramping toward 2.4GHz) while the first DMAs land
                warm = psA.tile([P, 256], F32, tag="warm", name="warm", bufs=1)
                for i in range(18):
                    nc.tensor.matmul(warm[:], lhsT=warm_sb[:, :P],
                                     rhs=warm_sb[:],
                                     start=(i == 0), stop=(i == 17))
                for f in range(KF):
                    # separate w1/w3 tiles: the w1-block matmuls depend only
                    # on the w1 half's DMA (per-tile dependency tracking)
                    # all wf issues ride the scalar engine, whose silu ops
                    # compute-pace them (engines execute in program order),
                    # so weight prefetch never floods the startup window.
                    # wfb0 goes via gpsimd (scalar is busy with wfa0).
                    wfa = wpool.tile([P, KH, P], BF16, tag="wfa", name="wfa",
                                     bufs=3)
                    wfb = wpool.tile([P, KH, P], BF16, tag="wfb", name="wfb",
                                     bufs=3)
                    nc.scalar.dma_start(out=wfa[:], in_=w13.ap()[:, f, 0])
                    eb = nc.gpsimd if f == 0 else nc.scalar
                    eb.dma_start(out=wfb[:], in_=w13.ap()[:, f, 1])
                    if f == 8:
                        nc.sync.dma_start(out=w2_s[0][:], in_=w2d.ap()[0])
                    pss = []
                    for i, (c0, c1) in enumerate(chunks):
                        ps1 = psA.tile([P, s], F32, tag=f"ps1_{i}",
                                       name=f"ps1_{i}")
                        for k in range(KH):
                            nc.tensor.matmul(ps1[:], lhsT=wfa[:, k, :],
                                             rhs=xg_s[i][:, k, :],
                                             start=(k == 0), stop=(k == KH - 1))
                        sil = evac.tile([P, s], F32, tag=f"sil_{i}",
                                        name=f"sil_{i}", bufs=1)
                        nc.scalar.activation(sil[:], ps1[:],
                                             mybir.ActivationFunctionType.Silu)
                        pss.append(sil)
                    for i, (c0, c1) in enumerate(chunks):
                        ps3 = psA.tile([P, s], F32, tag=f"ps3_{i}",
                                       name=f"ps3_{i}")
                        for k in range(KH):
                            nc.tensor.matmul(ps3[:], lhsT=wfb[:, k, :],
                                             rhs=xg_s[i][:, k, :],
                                             start=(k == 0), stop=(k == KH - 1))
                        nc.vector.tensor_tensor(inter[:, f, c0:c1],
                                                pss[i][:], ps3[:],
                                                op=mybir.AluOpType.mult)
                nc.sync.dma_start(out=w2_s[1][:], in_=w2d.ap()[1])

            # Phase B: oute[t, :] = cw[t] * (inter.T @ w2)[t, :]
            # n-outer so the n=1 half of w2 can stream during the n=0 pass;
            # per-m PSUM tags mean consecutive groups never share a bank, so
            # each [ms, 512] group evacs while the next group's matmuls run
            with tc.tile_pool(name="psB", bufs=1, space=PSUM) as psB:
                for n in range(H // 512):
                    for m, ms in enumerate(MS):
                        ps = psB.tile([P, 512], F32, tag=f"psbm{m}",
                                      name=f"psbm{m}")
                        for k in range(KF):
                            nc.tensor.matmul(
                                ps[:ms, :],
                                lhsT=inter[:, k, m * P:m * P + ms],
                                rhs=w2_s[n][:, k, :],
                                start=(k == 0), stop=(k == KF - 1))
                        o = evac.tile([P, 512], F32, tag="o", name="o")
                        nc.vector.tensor_scalar_mul(o[:ms, :], ps[:ms, :],
                                                    cw_s[:ms, m:m + 1])
                        # alternate queues; the last outputs go via the idle
                        # scalar queue so teardown is not gated on a backlog
                        seq = n * MT + m
                        eng = (nc.scalar if seq >= 2 * MT - 2 else
                               (nc.sync if seq % 2 == 0 else nc.gpsimd))
                        eng.dma_start(
                            out=oute.ap()[m * P:m * P + ms,
                                          n * 512:(n + 1) * 512],
                            in_=o[:ms, :])
    nc.compile()
    return nc


def kernel(hidden_states, gate_w, w1, w2, w3):
    in_maps = make_in_maps(hidden_states, gate_w, w1, w2, w3)
    nc = _NC_CACHE["nc"]
    res = run_bass_kernel_spmd(nc, in_maps, core_ids=list(range(E)),
                               trace=False)
    return assemble(res.results)


def make_in_maps(hidden_states, gate_w, w1, w2, w3):
    toks, wts = _route(hidden_states, gate_w)
    C = _cap(toks)
    _NC_CACHE["route"] = (toks, wts, C)
    if "nc" not in _NC_CACHE or _NC_CACHE.get("C") != C:
        _NC_CACHE["nc"] = build(C)
        _NC_CACHE["C"] = C
    MT = -(-C // P)
    x = np.asarray(hidden_states, np.float32)
    in_maps = []
    for e in range(E):
        n_e = len(toks[e])
        xpad = np.zeros((C, H), np.float32)
        xpad[:n_e] = x[toks[e]]
        xgn = np.ascontiguousarray(
            xpad.reshape(NCH, C // NCH, KH, P)
            .transpose(0, 3, 2, 1).astype(BF16NP))
        w1r = np.asarray(w1[e], np.float32).reshape(KH, P, KF, P)
        w3r = np.asarray(w3[e], np.float32).reshape(KH, P, KF, P)
        w13n = np.ascontiguousarray(
            np.stack([w1r, w3r], 0).transpose(2, 3, 0, 1, 4).astype(BF16NP))
        w2n = np.ascontiguousarray(
            np.asarray(w2[e], np.float32).reshape(KF, P, 2, H // 2)
            .transpose(2, 1, 0, 3).astype(BF16NP))
        wpad = np.zeros(MT * P, np.float32)
        wpad[:n_e] = wts[e]
        cwn = np.ascontiguousarray(wpad.reshape(MT, P).T)
        in_maps.append({"xg": xgn, "w13": w13n, "w2d": w2n, "cwd": cwn})
    return in_maps


def assemble(results):
    toks, _, _ = _NC_CACHE["route"]
    out = np.zeros((T, H), np.float32)
    for e in range(E):
        out[toks[e]] += results[e]["oute"][:len(toks[e])]
    return out
